# revision 50
# baseline (speedup 1.0000x reference)
"""Trainium2 Bass kernel for causal self-attention (B=2, T=2048, C=1024, H=16).

Sharding: tensor-parallel over heads x data-parallel over batch.
Each of the 8 cores handles one (batch b, head-group g) pair: b = core // 4,
g = core % 4, where a head group is 4 consecutive heads (heads 4g..4g+3).

Per-core pipeline (v2 — software-pipelined, PE-saturating):
  Ramp: ci-major qkv chains (k0/q0/k1/q1 for qc0 + v tb0..3) so the PE
        computes while the 4MB xT streams in.
  Attention per (pair, qc), one k-block per step, AV lagging S by one step:
        PE order: S(kb+1) | filler | AV(kb); exp(kb+1) on ACT overlaps.
        Both heads' S^T live in one [128,1024] PSUM tile -> single exp.
  l-broadcast for free: v_aug columns 64..127 are 1.0, so AV's PSUM rows
        64..127 hold the softmax denominator replicated across partitions;
        normalize = DVE reciprocal + multiply straight out of PSUM.
  Fillers: remaining qk/v chains (phase 1) and projection chunks (phase 2)
        are interleaved between S and AV to hide exp latency and keep the
        PE p-state at max clock.
  Output projection partials summed on the host (the TP all-reduce), plus
        b_proj.
"""

import numpy as np
from contextlib import ExitStack

import concourse.bass as bass
import concourse.tile as tile
from concourse import bacc, library_config, mybir
from concourse.bass import ts
from concourse.bass_utils import run_bass_kernel_spmd

F32 = mybir.dt.float32
F32R = mybir.dt.float32r
BF16 = mybir.dt.bfloat16
AF = mybir.ActivationFunctionType
PSUM = bass.MemorySpace.PSUM

B, T, C, H = 2, 2048, 1024, 16
HD = C // H              # 64
HPC = 4                  # heads per core
PAIRS = 2                # head pairs per core
CI = C // 128            # 8 contraction chunks
TB = T // 128            # 16 t-blocks
NQC = T // 512           # 4 q-chunks
N_CORES = 8

IO_DT = BF16
QKV_DT = BF16
P_DT = BF16


def _emit(tc, nc, xT_d, w1_d, wv_d, wp_d, out_d):
    ctx = ExitStack()
    with ctx:
        pers = ctx.enter_context(tc.tile_pool(name="pers", bufs=1))
        nc.gpsimd.load_library(library_config.attn)

        # ---------------- persistent SBUF ----------------
        xT_tiles = [pers.tile([128, T], IO_DT, name=f"xt{ci}") for ci in range(CI)]
        # separate tiles per weight block -> fine-grained DMA deps
        w_sb = [pers.tile([128, 1024], IO_DT, name=f"w{j}") for j in range(4)]
        wv_sb = pers.tile([128, 2048], IO_DT, name="wv")
        wp_sb = pers.tile([128, 2048], IO_DT, name="wp")
        # deps are tile-granular: split q/k/v/y into per-chunk tiles so each
        # consumer waits only on its own producer, not the newest write
        qT = [[pers.tile([128, 512], QKV_DT, name=f"qT{p}_{qc}")
               for qc in range(NQC)] for p in range(PAIRS)]
        kT = [[pers.tile([128, 512], QKV_DT, name=f"kT{p}_{qc}")
               for qc in range(NQC)] for p in range(PAIRS)]
        # v_aug per t-block: 4 heads x [64 v | 64 ones]
        vt = [pers.tile([128, 512], QKV_DT, name=f"vt{tb}") for tb in range(TB)]
        yT = [[pers.tile([128, 512], QKV_DT, name=f"yT{p}_{qc}")
               for qc in range(NQC)] for p in range(PAIRS)]
        mask_d = pers.tile([128, 128], P_DT, name="mask_d")

        # ---------------- DMAs (spread across sequencers) ----------------
        # ordered so the first ramp chain (wk0 + xT0) unblocks earliest
        dmas = [(w_sb[0], w1_d[:, 0:1024]), (xT_tiles[0], xT_d[ts(0, 128), :]),
                (w_sb[1], w1_d[:, 1024:2048]), (xT_tiles[1], xT_d[ts(1, 128), :]),
                (wv_sb, wv_d[:]), (xT_tiles[2], xT_d[ts(2, 128), :]),
                (w_sb[2], w1_d[:, 2048:3072]), (xT_tiles[3], xT_d[ts(3, 128), :]),
                (w_sb[3], w1_d[:, 3072:4096]), (xT_tiles[4], xT_d[ts(4, 128), :]),
                (xT_tiles[5], xT_d[ts(5, 128), :]),
                (xT_tiles[6], xT_d[ts(6, 128), :]),
                (xT_tiles[7], xT_d[ts(7, 128), :]), (wp_sb, wp_d[:])]
        # gpsimd (SWDGE) delivery is ~10us late — keep inputs on sync/scalar
        for i, (dst, src) in enumerate(dmas):
            (nc.sync, nc.scalar)[i % 2].dma_start(dst[:], src)
        xT_sb = [t[:] for t in xT_tiles]

        # ones columns of v_aug (the AV matmul then emits the softmax
        # denominator broadcast across PSUM partitions 64..127 for free)
        for tb in range(TB):
            nc.vector.memset(vt[tb][:], 1.0)
        # 0/1 causal mask for the diagonal 128-block: (q - k >= 0)
        mask_f = pers.tile([128, 128], F32, name="mask_f")
        nc.gpsimd.memset(mask_f[:], 1.0)
        nc.gpsimd.affine_select(
            out=mask_f[:], in_=mask_f[:],
            compare_op=mybir.AluOpType.is_ge, fill=0.0,
            base=0, channel_multiplier=-1, pattern=[[1, 128]],
        )
        nc.vector.tensor_copy(mask_d[:], mask_f[:])

        def v_copy(psv, tb, split=False):
            # [128, 4h x 64d] PSUM -> per-head v_aug cols 0..63
            # (2D copies: multi-dim strided dst APs silently fail on DVE)
            for h in range(HPC):
                eng_copy = (nc.scalar.copy if split and h >= 2
                            else nc.vector.tensor_copy)
                eng_copy(
                    vt[tb][:, h * 128: h * 128 + 64],
                    psv[:, ts(h, 64)],
                )

        # ---------------- ramp: ci-major qkv chains ----------------
        # pair-0 k/q chains for qc 0 AND 1, plus v for tb 0..3, interleaved
        # by ci so each xT tile is consumed as soon as its DMA lands.
        ramp_kq = [(kT[0][0], 0, 0), (qT[0][0], 1, 0),
                   (kT[0][1], 0, 1), (qT[0][1], 1, 1)]
        with tc.tile_pool(name="psR", bufs=1, space=PSUM) as psR:
            pr_kq = [psR.tile([128, 512], F32, tag=f"rkq{i}", name=f"rkq{i}")
                     for i in range(4)]
            pr_v = [psR.tile([128, 256], F32, tag=f"rv{t}", name=f"rv{t}")
                    for t in range(4)]
            for ci in range(CI):
                for i, (dst, j, qc) in enumerate(ramp_kq):
                    nc.tensor.matmul(
                        pr_kq[i][:], w_sb[j][:, ts(ci, 128)],
                        xT_sb[ci][:, ts(qc, 512)],
                        start=(ci == 0), stop=(ci == CI - 1),
                    )
                for t in range(4):
                    nc.tensor.matmul(
                        pr_v[t][:], xT_sb[ci][:, ts(t, 128)],
                        wv_sb[:, ts(ci, 256)],
                        start=(ci == 0), stop=(ci == CI - 1),
                    )
            # drain PSUM on ACT+DVE in parallel; unblock attn(0,0) S then AV
            nc.vector.tensor_copy(kT[0][0][:], pr_kq[0][:])
            nc.scalar.copy(qT[0][0][:], pr_kq[1][:])
            v_copy(pr_v[0], 0, split=True)
            nc.vector.tensor_copy(kT[0][1][:], pr_kq[2][:])
            nc.scalar.copy(qT[0][1][:], pr_kq[3][:])
            for t in range(1, 4):
                v_copy(pr_v[t], t, split=True)

        # ---------------- main pools ----------------
        with (
            tc.tile_pool(name="psS", bufs=2, space=PSUM) as psS,   # 4 banks
            tc.tile_pool(name="psY", bufs=1, space=PSUM) as psY,   # 2 banks
            tc.tile_pool(name="psF", bufs=2, space=PSUM) as psF,   # 2 banks
            tc.tile_pool(name="pP", bufs=6) as pP,
            tc.tile_pool(name="pN", bufs=3) as pN,
            tc.tile_pool(name="pO", bufs=2) as pO,
        ):
            # ---- filler units ----
            def qk_chain(p, qc, which):
                dst = (kT if which == "k" else qT)[p][qc]
                j = 2 * p + (0 if which == "k" else 1)
                ps = psF.tile([128, 512], F32, tag="f", name="fqk")
                for ci in range(CI):
                    nc.tensor.matmul(
                        ps[:], w_sb[j][:, ts(ci, 128)],
                        xT_sb[ci][:, ts(qc, 512)],
                        start=(ci == 0), stop=(ci == CI - 1),
                    )
                nc.vector.tensor_copy(dst[:], ps[:])

            def v_chain(tb):
                psv = psF.tile([128, 256], F32, tag="f", name="fv")
                for ci in range(CI):
                    nc.tensor.matmul(
                        psv[:], xT_sb[ci][:, ts(tb, 128)],
                        wv_sb[:, ts(ci, 256)],
                        start=(ci == 0), stop=(ci == CI - 1),
                    )
                v_copy(psv, tb)

            ot_tiles = {}

            def proj_chunk(tb, cc):
                po = psF.tile([128, 512], F32, tag="f", name="fpo")
                for p in range(PAIRS):
                    nc.tensor.matmul(
                        po[:], yT[p][tb // 4][:, ts(tb % 4, 128)],
                        wp_sb[:, p * 1024 + cc * 512: p * 1024 + cc * 512 + 512],
                        start=(p == 0), stop=(p == PAIRS - 1),
                    )
                if cc == 0:
                    ot_tiles[tb] = pO.tile([128, 1024], F32, tag="ot", name="ot")
                ot = ot_tiles[tb]
                (nc.vector.tensor_copy if cc == 0
                 else nc.scalar.copy)(ot[:, ts(cc, 512)], po[:])
                if cc == 1:
                    eng = (nc.scalar, nc.gpsimd, nc.sync)[tb % 3]
                    eng.dma_start(out_d[ts(tb, 128), :], ot[:])
                    del ot_tiles[tb]

            filler_queue = []

            def emit_filler(n=1):
                for _ in range(n):
                    if not filler_queue:
                        return
                    kind, args = filler_queue.pop(0)
                    if kind == "qk":
                        qk_chain(*args)
                    elif kind == "v":
                        v_chain(*args)
                    else:
                        proj_chunk(*args)

            # ---- attention for (pair, qc): AV lags S by one step ----
            def attn(p, qc, budget=0):
                nkb = 4 * qc + 4
                ypt = [psY.tile([128, 512], F32, tag=f"y{hh}", name=f"y{hh}")
                       for hh in (0, 1)]
                pts = {}

                def s_step(kb):
                    col = max(0, (kb - 4 * qc) * 128)
                    sps = psS.tile([128, 1024], F32, tag="sps", name="sps")
                    for hh in (0, 1):
                        off = hh * 64
                        nc.tensor.matmul(
                            sps[:, hh * 512 + col: hh * 512 + 512],
                            kT[p][kb // 4][off:off + 64, ts(kb % 4, 128)],
                            qT[p][qc][off:off + 64, col:512],
                            start=True, stop=True,
                        )
                    pt = pP.tile([128, 1024], P_DT, tag="pt", name="pt")
                    if col == 0:
                        nc.scalar.activation(pt[:], sps[:], AF.Exp)
                    else:
                        for hh in (0, 1):
                            nc.scalar.activation(
                                pt[:, hh * 512 + col: hh * 512 + 512],
                                sps[:, hh * 512 + col: hh * 512 + 512],
                                AF.Exp)
                    if kb >= 4 * qc:   # mask the diagonal 128-block
                        for hh in (0, 1):
                            nc.vector.tensor_mul(
                                pt[:, hh * 512 + col: hh * 512 + col + 128],
                                pt[:, hh * 512 + col: hh * 512 + col + 128],
                                mask_d[:],
                            )
                    pts[kb] = (pt, col)

                def av_step(kb):
                    pt, col = pts.pop(kb)
                    for hh in (0, 1):
                        h = 2 * p + hh
                        nc.tensor.matmul(
                            ypt[hh][:, col:512],
                            vt[kb][:, ts(h, 128)],
                            pt[:, hh * 512 + col: hh * 512 + 512],
                            start=(kb == 0), stop=(kb == nkb - 1),
                        )

                s_step(0)
                for kb in range(nkb):
                    if kb + 1 < nkb:
                        s_step(kb + 1)
                    # spread `budget` filler units across steps, front-loaded
                    emit_filler(budget * (kb + 2) // (nkb + 1)
                                - budget * (kb + 1) // (nkb + 1))
                    av_step(kb)
                # lazy normalize. Normal path: ONE [65,512] copy per head
                # (split DVE/ACT) frees the ypt bank ~0.7us after the last AV;
                # the l-extract / broadcast / reciprocal / multiply then run
                # off the critical path (the consuming proj trails by a qc).
                # Final qc: latency to yT gates the drain, so broadcast l via
                # a PE outer product (the PE is idle right then) and multiply
                # straight out of PSUM.
                last = (p == 1 and qc == NQC - 1)
                if not last:
                    stage = []
                    for hh in (0, 1):
                        st = pN.tile([65, 512], F32, tag=f"st{hh}", name="st")
                        (nc.vector.tensor_copy if hh == 0
                         else nc.scalar.copy)(st[:], ypt[hh][0:65, :])
                        stage.append(st)
                    for hh in (0, 1):
                        st = stage[hh]
                        l_sb = pN.tile([1, 512], F32, tag="l", name="l_sb")
                        nc.scalar.copy(l_sb[:], st[64:65, :])
                        lb = pN.tile([64, 512], F32, tag="lb", name="lb")
                        nc.gpsimd.partition_broadcast(lb[:], l_sb[:])
                        rl = pN.tile([64, 512], F32, tag="rl", name="rl")
                        nc.vector.reciprocal_approx_fast(rl[:], lb[:])
                        nc.vector.tensor_mul(
                            yT[p][qc][hh * 64: hh * 64 + 64, :],
                            st[0:64, :], rl[:],
                        )
                else:
                    for hh in (0, 1):
                        l_sb = pN.tile([1, 512], F32, tag="l", name="l_sb")
                        nc.scalar.copy(l_sb[:], ypt[hh][64:65, :])
                        lb = pN.tile([64, 512], F32, tag="lb", name="lb")
                        nc.gpsimd.partition_broadcast(lb[:], l_sb[:])
                        rl = pN.tile([64, 512], F32, tag="rl", name="rl")
                        nc.vector.reciprocal_approx_fast(rl[:], lb[:])
                        nc.vector.tensor_mul(
                            yT[p][qc][hh * 64: hh * 64 + 64, :],
                            ypt[hh][0:64, :], rl[:],
                        )

            # ---- phase 1: pair 0 attention; fillers = v + pair-1 qk ----
            # queue order must respect deps: v(tb) before AV step kb=tb of
            # attn(0, tb//4); k1/q1(qc) anytime before attn(1, qc).
            filler_queue += [("qk", (1, 0, "k")), ("qk", (1, 0, "q")),
                             ("v", (4,))]                               # qc0: 3
            filler_queue += [("v", (5,)), ("v", (6,)), ("v", (7,)),
                             ("v", (8,)), ("v", (9,))]                  # qc1: 5
            filler_queue += [("v", (10,)), ("v", (11,)),
                             ("qk", (1, 1, "k")), ("qk", (1, 1, "q")),
                             ("v", (12,)), ("v", (13,))]                # qc2: 6
            filler_queue += [("v", (14,)), ("v", (15,)),
                             ("qk", (1, 2, "k")), ("qk", (1, 2, "q"))]  # qc3: 4
            p1_budget = [3, 5, 6, 4]

            for qc in range(NQC):
                if qc > 1:   # qc 0/1 chains were produced by the ramp
                    qk_chain(0, qc, "k")
                    qk_chain(0, qc, "q")
                attn(0, qc, p1_budget[qc])
            # phase-transition cover for attn(1,0)'s PSUM-bank reuse
            qk_chain(1, 3, "k")
            qk_chain(1, 3, "q")

            # ---- phase 2: pair 1 attention; fillers = projection ----
            # budgets keep 2 chunks of each qc for the NEXT attention so the
            # first pops at a boundary never wait on the just-written yT
            p2_budget = [0, 6, 8, 10]
            for qc in range(NQC):
                attn(1, qc, p2_budget[qc])
                filler_queue += [("proj", (tb, cc))
                                 for tb in range(4 * qc, 4 * qc + 4)
                                 for cc in range(2)]
            emit_filler(len(filler_queue))



_NC_CACHE = None


def _build():
    global _NC_CACHE
    if _NC_CACHE is not None:
        return _NC_CACHE
    nc = bacc.Bacc("TRN2", target_bir_lowering=False, debug=False,
                   num_devices=N_CORES)
    xT_d = nc.dram_tensor("xT", [C, T], IO_DT, kind="ExternalInput")
    w1_d = nc.dram_tensor("w1", [128, 4096], IO_DT, kind="ExternalInput")
    wv_d = nc.dram_tensor("wv", [128, 2048], IO_DT, kind="ExternalInput")
    wp_d = nc.dram_tensor("wp", [128, 2048], IO_DT, kind="ExternalInput")
    out_d = nc.dram_tensor("out", [T, C], F32, kind="ExternalOutput")

    with tile.TileContext(nc) as tc:
        _emit(tc, nc, xT_d, w1_d, wv_d, wp_d, out_d)
    nc.compile()
    _NC_CACHE = nc
    return nc


def _pack_pair(m):
    # [1024, 128] -> lhsT chunks layout [128, 8*128]
    return np.ascontiguousarray(
        m.reshape(CI, 128, 128).transpose(1, 0, 2).reshape(128, 1024))


def _io_np(a):
    import ml_dtypes
    return np.ascontiguousarray(a.astype(ml_dtypes.bfloat16))


def _in_maps(x, w_attn, w_proj):
    x = np.asarray(x, dtype=np.float32)
    w_attn = np.asarray(w_attn, dtype=np.float32)
    w_proj = np.asarray(w_proj, dtype=np.float32)
    xT = [_io_np(x[b].T) for b in range(B)]
    maps = []
    for core in range(N_CORES):
        b, g = core // HPC, core % HPC
        cols = slice(g * 256, (g + 1) * 256)
        wk_full = w_attn[:, 0 * C:1 * C][:, cols]
        wq_full = w_attn[:, 1 * C:2 * C][:, cols] * np.float32(1.0 / np.sqrt(HD))
        wv_full = w_attn[:, 2 * C:3 * C][:, cols]
        w1 = np.concatenate(
            [_pack_pair(m[:, p * 128:(p + 1) * 128])
             for p in range(PAIRS) for m in (wk_full, wq_full)], axis=1)
        wv_in = wv_full.reshape(CI, 128, 256).transpose(1, 0, 2).reshape(128, 2048)
        wp_in = (w_proj[g * 256:(g + 1) * 256, :]
                 .reshape(PAIRS, 128, 1024).transpose(1, 0, 2).reshape(128, 2048))
        maps.append({"xT": xT[b], "w1": _io_np(w1),
                     "wv": _io_np(wv_in), "wp": _io_np(wp_in)})
    return maps


def _assemble(results, b_proj):
    b_proj = np.asarray(b_proj, dtype=np.float32)
    out = np.zeros((B, T, C), dtype=np.float32)
    for core in range(N_CORES):
        out[core // HPC] += results[core]["out"]
    out += b_proj[None, None, :]
    return out


def kernel(x, w_attn, w_proj, b_proj):
    nc = _build()
    maps = _in_maps(x, w_attn, w_proj)
    res = run_bass_kernel_spmd(nc, maps, list(range(N_CORES)))
    return _assemble(res.results, b_proj)


def kernel_traced(x, w_attn, w_proj, b_proj):
    """Like kernel() but with NTFF tracing; returns (out, BassKernelResults)."""
    nc = _build()
    maps = _in_maps(x, w_attn, w_proj)
    res = run_bass_kernel_spmd(nc, maps, list(range(N_CORES)), trace=True)
    return _assemble(res.results, b_proj), res


# revision 54
# speedup vs baseline: 1.0283x; 1.0283x over previous
"""Trainium2 Bass kernel for causal self-attention (B=2, T=2048, C=1024, H=16).

Sharding: tensor-parallel over heads x data-parallel over batch.
Each of the 8 cores handles one (batch b, head-group g) pair: b = core // 4,
g = core % 4, where a head group is 4 consecutive heads (heads 4g..4g+3).

Per-core pipeline (v2 — software-pipelined, PE-saturating):
  Ramp: ci-major qkv chains (k0/q0/k1/q1 for qc0 + v tb0..3) so the PE
        computes while the 4MB xT streams in.
  Attention per (pair, qc), one k-block per step, AV lagging S by one step:
        PE order: S(kb+1) | filler | AV(kb); exp(kb+1) on ACT overlaps.
        Both heads' S^T live in one [128,1024] PSUM tile -> single exp.
  l-broadcast for free: v_aug columns 64..127 are 1.0, so AV's PSUM rows
        64..127 hold the softmax denominator replicated across partitions;
        normalize = DVE reciprocal + multiply straight out of PSUM.
  Fillers: remaining qk/v chains (phase 1) and projection chunks (phase 2)
        are interleaved between S and AV to hide exp latency and keep the
        PE p-state at max clock.
  Output projection partials summed on the host (the TP all-reduce), plus
        b_proj.
"""

import numpy as np
from contextlib import ExitStack

import concourse.bass as bass
import concourse.tile as tile
from concourse import bacc, library_config, mybir
from concourse.bass import ts
from concourse.bass_utils import run_bass_kernel_spmd

F32 = mybir.dt.float32
F32R = mybir.dt.float32r
BF16 = mybir.dt.bfloat16
AF = mybir.ActivationFunctionType
PSUM = bass.MemorySpace.PSUM

B, T, C, H = 2, 2048, 1024, 16
HD = C // H              # 64
HPC = 4                  # heads per core
PAIRS = 2                # head pairs per core
CI = C // 128            # 8 contraction chunks
TB = T // 128            # 16 t-blocks
NQC = T // 512           # 4 q-chunks
N_CORES = 8

IO_DT = BF16
QKV_DT = BF16
P_DT = BF16


def _emit(tc, nc, xT_d, w1_d, wv_d, wp_d, out_d):
    ctx = ExitStack()
    with ctx:
        pers = ctx.enter_context(tc.tile_pool(name="pers", bufs=1))
        nc.gpsimd.load_library(library_config.attn)

        # ---------------- persistent SBUF ----------------
        xT_tiles = [pers.tile([128, T], IO_DT, name=f"xt{ci}") for ci in range(CI)]
        # separate tiles per weight block -> fine-grained DMA deps
        w_sb = [pers.tile([128, 1024], IO_DT, name=f"w{j}") for j in range(4)]
        wv_sb = pers.tile([128, 2048], IO_DT, name="wv")
        wp_sb = pers.tile([128, 2048], IO_DT, name="wp")
        # deps are tile-granular: split q/k/v/y into per-chunk tiles so each
        # consumer waits only on its own producer, not the newest write
        qT = [[pers.tile([128, 512], QKV_DT, name=f"qT{p}_{qc}")
               for qc in range(NQC)] for p in range(PAIRS)]
        kT = [[pers.tile([128, 512], QKV_DT, name=f"kT{p}_{qc}")
               for qc in range(NQC)] for p in range(PAIRS)]
        # v_aug per t-block: 4 heads x [64 v | 64 ones]
        vt = [pers.tile([128, 512], QKV_DT, name=f"vt{tb}") for tb in range(TB)]
        yT = [[pers.tile([128, 512], QKV_DT, name=f"yT{p}_{qc}")
               for qc in range(NQC)] for p in range(PAIRS)]
        mask_d = pers.tile([128, 128], P_DT, name="mask_d")

        # ---------------- DMAs (spread across sequencers) ----------------
        # ordered so the first ramp chain (wk0 + xT0) unblocks earliest
        dmas = [(w_sb[0], w1_d[:, 0:1024]), (xT_tiles[0], xT_d[ts(0, 128), :]),
                (w_sb[1], w1_d[:, 1024:2048]), (xT_tiles[1], xT_d[ts(1, 128), :]),
                (wv_sb, wv_d[:]), (xT_tiles[2], xT_d[ts(2, 128), :]),
                (w_sb[2], w1_d[:, 2048:3072]), (xT_tiles[3], xT_d[ts(3, 128), :]),
                (w_sb[3], w1_d[:, 3072:4096]), (xT_tiles[4], xT_d[ts(4, 128), :]),
                (xT_tiles[5], xT_d[ts(5, 128), :]),
                (xT_tiles[6], xT_d[ts(6, 128), :]),
                (xT_tiles[7], xT_d[ts(7, 128), :]), (wp_sb, wp_d[:])]
        # gpsimd (SWDGE) delivery is ~10us late — keep inputs on sync/scalar
        for i, (dst, src) in enumerate(dmas):
            (nc.sync, nc.scalar)[i % 2].dma_start(dst[:], src)
        xT_sb = [t[:] for t in xT_tiles]

        # ones columns of v_aug (the AV matmul then emits the softmax
        # denominator broadcast across PSUM partitions 64..127 for free)
        for tb in range(TB):
            nc.gpsimd.memset(vt[tb][:], 1.0)
        # 0/1 causal mask for the diagonal 128-block: (q - k >= 0)
        mask_f = pers.tile([128, 128], F32, name="mask_f")
        nc.gpsimd.memset(mask_f[:], 1.0)
        nc.gpsimd.affine_select(
            out=mask_f[:], in_=mask_f[:],
            compare_op=mybir.AluOpType.is_ge, fill=0.0,
            base=0, channel_multiplier=-1, pattern=[[1, 128]],
        )
        nc.vector.tensor_copy(mask_d[:], mask_f[:])

        def v_copy(psv, tb, split=False):
            # [128, 4h x 64d] PSUM -> per-head v_aug cols 0..63
            # (2D copies: multi-dim strided dst APs silently fail on DVE)
            for h in range(HPC):
                eng_copy = (nc.scalar.copy if split and h >= 2
                            else nc.vector.tensor_copy)
                eng_copy(
                    vt[tb][:, h * 128: h * 128 + 64],
                    psv[:, ts(h, 64)],
                )

        # ---------------- main pools ----------------
        with (
            tc.tile_pool(name="psS", bufs=2, space=PSUM) as psS,   # 4 banks
            tc.tile_pool(name="psY", bufs=1, space=PSUM) as psY,   # 2 banks
            tc.tile_pool(name="psF", bufs=2, space=PSUM) as psF,   # 2 banks
            tc.tile_pool(name="pP", bufs=6) as pP,
            tc.tile_pool(name="pN", bufs=3) as pN,
            tc.tile_pool(name="pO", bufs=2) as pO,
        ):
            # ---- ramp: ci-major qkv chains ----
            # pair-0 k/q chains for qc 0 AND 1, plus v for tb 0..3,
            # interleaved by ci so each xT tile is consumed as soon as its
            # DMA lands. Accumulators live in the MAIN pools' tag rings (a
            # separate pool's release would barrier the first attention
            # writes on ALL ramp drain copies).
            rkq = [psS.tile([128, 1024], F32, tag="sps", name=f"rkq{i}")
                   for i in range(2)]
            rv = [psY.tile([128, 512], F32, tag="y0", name="rv0"),
                  psY.tile([128, 512], F32, tag="y1", name="rv1"),
                  psF.tile([128, 512], F32, tag="f", name="rv2"),
                  psF.tile([128, 512], F32, tag="f", name="rv3")]
            ramp_kq = [(kT[0][0], 0, 0), (qT[0][0], 1, 0),
                       (kT[0][1], 0, 1), (qT[0][1], 1, 1)]
            for ci in range(CI):
                for i, (dst, j, qc) in enumerate(ramp_kq):
                    nc.tensor.matmul(
                        rkq[i // 2][:, ts(i % 2, 512)],
                        w_sb[j][:, ts(ci, 128)],
                        xT_sb[ci][:, ts(qc, 512)],
                        start=(ci == 0), stop=(ci == CI - 1),
                    )
                for t in range(4):
                    nc.tensor.matmul(
                        rv[t][:, 0:256], xT_sb[ci][:, ts(t, 128)],
                        wv_sb[:, ts(ci, 256)],
                        start=(ci == 0), stop=(ci == CI - 1),
                    )
            # drain PSUM on ACT+DVE in parallel; unblock attn(0,0) S then AV
            nc.vector.tensor_copy(kT[0][0][:], rkq[0][:, 0:512])
            nc.scalar.copy(qT[0][0][:], rkq[0][:, 512:1024])
            v_copy(rv[0], 0, split=True)
            nc.vector.tensor_copy(kT[0][1][:], rkq[1][:, 0:512])
            nc.scalar.copy(qT[0][1][:], rkq[1][:, 512:1024])
            for t in range(1, 4):
                v_copy(rv[t], t, split=True)
            # ---- filler units ----
            def qk_chain(p, qc, which):
                dst = (kT if which == "k" else qT)[p][qc]
                j = 2 * p + (0 if which == "k" else 1)
                ps = psF.tile([128, 512], F32, tag="f", name="fqk")
                for ci in range(CI):
                    nc.tensor.matmul(
                        ps[:], w_sb[j][:, ts(ci, 128)],
                        xT_sb[ci][:, ts(qc, 512)],
                        start=(ci == 0), stop=(ci == CI - 1),
                    )
                nc.vector.tensor_copy(dst[:], ps[:])

            def v_chain(tb):
                psv = psF.tile([128, 256], F32, tag="f", name="fv")
                for ci in range(CI):
                    nc.tensor.matmul(
                        psv[:], xT_sb[ci][:, ts(tb, 128)],
                        wv_sb[:, ts(ci, 256)],
                        start=(ci == 0), stop=(ci == CI - 1),
                    )
                v_copy(psv, tb)

            ot_tiles = {}

            def proj_chunk(tb, cc):
                po = psF.tile([128, 512], F32, tag="f", name="fpo")
                for p in range(PAIRS):
                    nc.tensor.matmul(
                        po[:], yT[p][tb // 4][:, ts(tb % 4, 128)],
                        wp_sb[:, p * 1024 + cc * 512: p * 1024 + cc * 512 + 512],
                        start=(p == 0), stop=(p == PAIRS - 1),
                    )
                if cc == 0:
                    ot_tiles[tb] = pO.tile([128, 1024], F32, tag="ot", name="ot")
                ot = ot_tiles[tb]
                nc.vector.tensor_copy(ot[:, ts(cc, 512)], po[:])
                if cc == 1:
                    eng = (nc.scalar, nc.gpsimd, nc.sync)[tb % 3]
                    eng.dma_start(out_d[ts(tb, 128), :], ot[:])
                    del ot_tiles[tb]

            filler_queue = []

            def emit_filler(n=1):
                for _ in range(n):
                    if not filler_queue:
                        return
                    kind, args = filler_queue.pop(0)
                    if kind == "qk":
                        qk_chain(*args)
                    elif kind == "v":
                        v_chain(*args)
                    else:
                        proj_chunk(*args)

            # ---- attention for (pair, qc): AV lags S by one step ----
            def attn(p, qc, budget=0):
                nkb = 4 * qc + 4
                ypt = [psY.tile([128, 512], F32, tag=f"y{hh}", name=f"y{hh}")
                       for hh in (0, 1)]
                pts = {}

                def s_step(kb):
                    col = max(0, (kb - 4 * qc) * 128)
                    sps = psS.tile([128, 1024], F32, tag="sps", name="sps")
                    for hh in (0, 1):
                        off = hh * 64
                        nc.tensor.matmul(
                            sps[:, hh * 512 + col: hh * 512 + 512],
                            kT[p][kb // 4][off:off + 64, ts(kb % 4, 128)],
                            qT[p][qc][off:off + 64, col:512],
                            start=True, stop=True,
                        )
                    pt = pP.tile([128, 1024], P_DT, tag="pt", name="pt")
                    if col == 0:
                        nc.scalar.activation(pt[:], sps[:], AF.Exp)
                    else:
                        for hh in (0, 1):
                            nc.scalar.activation(
                                pt[:, hh * 512 + col: hh * 512 + 512],
                                sps[:, hh * 512 + col: hh * 512 + 512],
                                AF.Exp)
                    if kb >= 4 * qc:   # mask the diagonal 128-block
                        for hh in (0, 1):
                            nc.vector.tensor_mul(
                                pt[:, hh * 512 + col: hh * 512 + col + 128],
                                pt[:, hh * 512 + col: hh * 512 + col + 128],
                                mask_d[:],
                            )
                    pts[kb] = (pt, col)

                def av_step(kb):
                    pt, col = pts.pop(kb)
                    for hh in (0, 1):
                        h = 2 * p + hh
                        nc.tensor.matmul(
                            ypt[hh][:, col:512],
                            vt[kb][:, ts(h, 128)],
                            pt[:, hh * 512 + col: hh * 512 + 512],
                            start=(kb == 0), stop=(kb == nkb - 1),
                        )

                s_step(0)
                # spread `budget` fillers across steps, front-loaded, leaving
                # the last two steps clear so DVE/ACT drain by the boundary
                nfs = max(nkb - 2, 1)
                for kb in range(nkb):
                    if kb + 1 < nkb:
                        s_step(kb + 1)
                    if kb < nfs:
                        emit_filler(budget * (kb + 2) // (nfs + 1)
                                    - budget * (kb + 1) // (nfs + 1))
                    av_step(kb)
                # lazy normalize. Normal path: ONE [65,512] copy per head
                # (split DVE/ACT) frees the ypt bank ~0.7us after the last AV;
                # the l-extract / broadcast / reciprocal / multiply then run
                # off the critical path (the consuming proj trails by a qc).
                # Final qc: latency to yT gates the drain, so broadcast l via
                # a PE outer product (the PE is idle right then) and multiply
                # straight out of PSUM.
                last = (p == 1 and qc == NQC - 1)
                if not last:
                    stage = []
                    for hh in (0, 1):
                        st = pN.tile([65, 512], F32, tag=f"st{hh}", name="st")
                        (nc.vector.tensor_copy if hh == 0
                         else nc.scalar.copy)(st[:], ypt[hh][0:65, :])
                        stage.append(st)
                    for hh in (0, 1):
                        st = stage[hh]
                        l_sb = pN.tile([1, 512], F32, tag="l", name="l_sb")
                        nc.scalar.copy(l_sb[:], st[64:65, :])
                        lb = pN.tile([64, 512], F32, tag="lb", name="lb")
                        nc.gpsimd.partition_broadcast(lb[:], l_sb[:])
                        rl = pN.tile([64, 512], F32, tag="rl", name="rl")
                        nc.vector.reciprocal_approx_fast(rl[:], lb[:])
                        nc.vector.tensor_mul(
                            yT[p][qc][hh * 64: hh * 64 + 64, :],
                            st[0:64, :], rl[:],
                        )
                else:
                    for hh in (0, 1):
                        l_sb = pN.tile([1, 512], F32, tag="l", name="l_sb")
                        nc.scalar.copy(l_sb[:], ypt[hh][64:65, :])
                        lb = pN.tile([64, 512], F32, tag="lb", name="lb")
                        nc.gpsimd.partition_broadcast(lb[:], l_sb[:])
                        rl = pN.tile([64, 512], F32, tag="rl", name="rl")
                        nc.vector.reciprocal_approx_fast(rl[:], lb[:])
                        nc.vector.tensor_mul(
                            yT[p][qc][hh * 64: hh * 64 + 64, :],
                            ypt[hh][0:64, :], rl[:],
                        )

            # ---- phase 1: pair 0 attention; fillers = v + pair-1 qk ----
            # queue order must respect deps: v(tb) before AV step kb=tb of
            # attn(0, tb//4); k1/q1(qc) anytime before attn(1, qc).
            filler_queue += [("qk", (1, 0, "k")), ("qk", (1, 0, "q")),
                             ("v", (4,))]                               # qc0: 3
            filler_queue += [("v", (5,)), ("v", (6,)), ("v", (7,)),
                             ("v", (8,)), ("v", (9,))]                  # qc1: 5
            filler_queue += [("v", (10,)), ("v", (11,)),
                             ("qk", (1, 1, "k")), ("qk", (1, 1, "q")),
                             ("v", (12,)), ("v", (13,))]                # qc2: 6
            filler_queue += [("v", (14,)), ("v", (15,)),
                             ("qk", (1, 2, "k")), ("qk", (1, 2, "q"))]  # qc3: 4
            p1_budget = [3, 5, 6, 4]

            for qc in range(NQC):
                if qc > 1:   # qc 0/1 chains were produced by the ramp
                    qk_chain(0, qc, "k")
                    qk_chain(0, qc, "q")
                attn(0, qc, p1_budget[qc])
            # phase-transition cover for attn(1,0)'s PSUM-bank reuse
            qk_chain(1, 3, "k")
            qk_chain(1, 3, "q")

            # ---- phase 2: pair 1 attention; fillers = projection ----
            # budgets keep 2 chunks of each qc for the NEXT attention so the
            # first pops at a boundary never wait on the just-written yT
            p2_budget = [0, 6, 8, 10]
            for qc in range(NQC):
                attn(1, qc, p2_budget[qc])
                filler_queue += [("proj", (tb, cc))
                                 for tb in range(4 * qc, 4 * qc + 4)
                                 for cc in range(2)]
            emit_filler(len(filler_queue))



_NC_CACHE = None


def _build():
    global _NC_CACHE
    if _NC_CACHE is not None:
        return _NC_CACHE
    nc = bacc.Bacc("TRN2", target_bir_lowering=False, debug=False,
                   num_devices=N_CORES)
    xT_d = nc.dram_tensor("xT", [C, T], IO_DT, kind="ExternalInput")
    w1_d = nc.dram_tensor("w1", [128, 4096], IO_DT, kind="ExternalInput")
    wv_d = nc.dram_tensor("wv", [128, 2048], IO_DT, kind="ExternalInput")
    wp_d = nc.dram_tensor("wp", [128, 2048], IO_DT, kind="ExternalInput")
    out_d = nc.dram_tensor("out", [T, C], F32, kind="ExternalOutput")

    with tile.TileContext(nc) as tc:
        _emit(tc, nc, xT_d, w1_d, wv_d, wp_d, out_d)
    nc.compile()
    _NC_CACHE = nc
    return nc


def _pack_pair(m):
    # [1024, 128] -> lhsT chunks layout [128, 8*128]
    return np.ascontiguousarray(
        m.reshape(CI, 128, 128).transpose(1, 0, 2).reshape(128, 1024))


def _io_np(a):
    import ml_dtypes
    return np.ascontiguousarray(a.astype(ml_dtypes.bfloat16))


def _in_maps(x, w_attn, w_proj):
    x = np.asarray(x, dtype=np.float32)
    w_attn = np.asarray(w_attn, dtype=np.float32)
    w_proj = np.asarray(w_proj, dtype=np.float32)
    xT = [_io_np(x[b].T) for b in range(B)]
    maps = []
    for core in range(N_CORES):
        b, g = core // HPC, core % HPC
        cols = slice(g * 256, (g + 1) * 256)
        wk_full = w_attn[:, 0 * C:1 * C][:, cols]
        wq_full = w_attn[:, 1 * C:2 * C][:, cols] * np.float32(1.0 / np.sqrt(HD))
        wv_full = w_attn[:, 2 * C:3 * C][:, cols]
        w1 = np.concatenate(
            [_pack_pair(m[:, p * 128:(p + 1) * 128])
             for p in range(PAIRS) for m in (wk_full, wq_full)], axis=1)
        wv_in = wv_full.reshape(CI, 128, 256).transpose(1, 0, 2).reshape(128, 2048)
        wp_in = (w_proj[g * 256:(g + 1) * 256, :]
                 .reshape(PAIRS, 128, 1024).transpose(1, 0, 2).reshape(128, 2048))
        maps.append({"xT": xT[b], "w1": _io_np(w1),
                     "wv": _io_np(wv_in), "wp": _io_np(wp_in)})
    return maps


def _assemble(results, b_proj):
    b_proj = np.asarray(b_proj, dtype=np.float32)
    out = np.zeros((B, T, C), dtype=np.float32)
    for core in range(N_CORES):
        out[core // HPC] += results[core]["out"]
    out += b_proj[None, None, :]
    return out


def kernel(x, w_attn, w_proj, b_proj):
    nc = _build()
    maps = _in_maps(x, w_attn, w_proj)
    res = run_bass_kernel_spmd(nc, maps, list(range(N_CORES)))
    return _assemble(res.results, b_proj)


def kernel_traced(x, w_attn, w_proj, b_proj):
    """Like kernel() but with NTFF tracing; returns (out, BassKernelResults)."""
    nc = _build()
    maps = _in_maps(x, w_attn, w_proj)
    res = run_bass_kernel_spmd(nc, maps, list(range(N_CORES)), trace=True)
    return _assemble(res.results, b_proj), res


# revision 56
# speedup vs baseline: 1.0569x; 1.0278x over previous
"""Trainium2 Bass kernel for causal self-attention (B=2, T=2048, C=1024, H=16).

Sharding: tensor-parallel over heads x data-parallel over batch.
Each of the 8 cores handles one (batch b, head-group g) pair: b = core // 4,
g = core % 4, where a head group is 4 consecutive heads (heads 4g..4g+3).

Per-core pipeline (v2 — software-pipelined, PE-saturating):
  Ramp: ci-major qkv chains (k0/q0/k1/q1 for qc0 + v tb0..3) so the PE
        computes while the 4MB xT streams in.
  Attention per (pair, qc), one k-block per step, AV lagging S by one step:
        PE order: S(kb+1) | filler | AV(kb); exp(kb+1) on ACT overlaps.
        Both heads' S^T live in one [128,1024] PSUM tile -> single exp.
  l-broadcast for free: v_aug columns 64..127 are 1.0, so AV's PSUM rows
        64..127 hold the softmax denominator replicated across partitions;
        normalize = DVE reciprocal + multiply straight out of PSUM.
  Fillers: remaining qk/v chains (phase 1) and projection chunks (phase 2)
        are interleaved between S and AV to hide exp latency and keep the
        PE p-state at max clock.
  Output projection partials summed on the host (the TP all-reduce), plus
        b_proj.
"""

import numpy as np
from contextlib import ExitStack

import concourse.bass as bass
import concourse.tile as tile
from concourse import bacc, library_config, mybir
from concourse.bass import ts
from concourse.bass_utils import run_bass_kernel_spmd

F32 = mybir.dt.float32
F32R = mybir.dt.float32r
BF16 = mybir.dt.bfloat16
AF = mybir.ActivationFunctionType
PSUM = bass.MemorySpace.PSUM

B, T, C, H = 2, 2048, 1024, 16
HD = C // H              # 64
HPC = 4                  # heads per core
PAIRS = 2                # head pairs per core
CI = C // 128            # 8 contraction chunks
TB = T // 128            # 16 t-blocks
NQC = T // 512           # 4 q-chunks
N_CORES = 8

IO_DT = BF16
QKV_DT = BF16
P_DT = BF16


def _emit(tc, nc, xT_d, w1_d, wv_d, wp_d, out_d):
    ctx = ExitStack()
    with ctx:
        pers = ctx.enter_context(tc.tile_pool(name="pers", bufs=1))
        nc.gpsimd.load_library(library_config.attn)

        # ---------------- persistent SBUF ----------------
        xT_tiles = [pers.tile([128, T], IO_DT, name=f"xt{ci}") for ci in range(CI)]
        # separate tiles per weight block -> fine-grained DMA deps
        w_sb = [pers.tile([128, 1024], IO_DT, name=f"w{j}") for j in range(4)]
        wv_sb = pers.tile([128, 2048], IO_DT, name="wv")
        wp_sb = pers.tile([128, 2048], IO_DT, name="wp")
        # deps are tile-granular: split q/k/v/y into per-chunk tiles so each
        # consumer waits only on its own producer, not the newest write
        qT = [[pers.tile([128, 512], QKV_DT, name=f"qT{p}_{qc}")
               for qc in range(NQC)] for p in range(PAIRS)]
        kT = [[pers.tile([128, 512], QKV_DT, name=f"kT{p}_{qc}")
               for qc in range(NQC)] for p in range(PAIRS)]
        # v_aug per t-block: 4 heads x [64 v | 64 ones]
        vt = [pers.tile([128, 512], QKV_DT, name=f"vt{tb}") for tb in range(TB)]
        yT = [[pers.tile([128, 512], QKV_DT, name=f"yT{p}_{qc}")
               for qc in range(NQC)] for p in range(PAIRS)]
        mask_d = pers.tile([128, 128], P_DT, name="mask_d")

        # ---------------- DMAs (spread across sequencers) ----------------
        # ordered so the first ramp chain (wk0 + xT0) unblocks earliest
        dmas = [(w_sb[0], w1_d[:, 0:1024]), (xT_tiles[0], xT_d[ts(0, 128), :]),
                (w_sb[1], w1_d[:, 1024:2048]), (xT_tiles[1], xT_d[ts(1, 128), :]),
                (wv_sb, wv_d[:]), (xT_tiles[2], xT_d[ts(2, 128), :]),
                (w_sb[2], w1_d[:, 2048:3072]), (xT_tiles[3], xT_d[ts(3, 128), :]),
                (w_sb[3], w1_d[:, 3072:4096]), (xT_tiles[4], xT_d[ts(4, 128), :]),
                (xT_tiles[5], xT_d[ts(5, 128), :]),
                (xT_tiles[6], xT_d[ts(6, 128), :]),
                (xT_tiles[7], xT_d[ts(7, 128), :]), (wp_sb, wp_d[:])]
        # gpsimd (SWDGE) delivery is ~10us late — keep inputs on sync/scalar
        for i, (dst, src) in enumerate(dmas):
            (nc.sync, nc.scalar)[i % 2].dma_start(dst[:], src)
        xT_sb = [t[:] for t in xT_tiles]

        # ones columns of v_aug (the AV matmul then emits the softmax
        # denominator broadcast across PSUM partitions 64..127 for free)
        for tb in range(TB):
            nc.gpsimd.memset(vt[tb][:], 1.0)
        # 0/1 causal mask for the diagonal 128-block: (q - k >= 0)
        mask_f = pers.tile([128, 128], F32, name="mask_f")
        nc.gpsimd.memset(mask_f[:], 1.0)
        nc.gpsimd.affine_select(
            out=mask_f[:], in_=mask_f[:],
            compare_op=mybir.AluOpType.is_ge, fill=0.0,
            base=0, channel_multiplier=-1, pattern=[[1, 128]],
        )
        nc.vector.tensor_copy(mask_d[:], mask_f[:])

        def v_copy(psv, tb, split=False):
            # [128, 4h x 64d] PSUM -> per-head v_aug cols 0..63
            # (2D copies: multi-dim strided dst APs silently fail on DVE)
            for h in range(HPC):
                eng_copy = (nc.scalar.copy if split and h >= 2
                            else nc.vector.tensor_copy)
                eng_copy(
                    vt[tb][:, h * 128: h * 128 + 64],
                    psv[:, ts(h, 64)],
                )

        # ---------------- main pools ----------------
        with (
            tc.tile_pool(name="psS", bufs=2, space=PSUM) as psS,   # 4 banks
            tc.tile_pool(name="psY", bufs=1, space=PSUM) as psY,   # 2 banks
            tc.tile_pool(name="psF", bufs=2, space=PSUM) as psF,   # 2 banks
            tc.tile_pool(name="pP", bufs=6) as pP,
            tc.tile_pool(name="pN", bufs=3) as pN,
            tc.tile_pool(name="pO", bufs=2) as pO,
        ):
            # ---- ramp: ci-major qkv chains ----
            # pair-0 k/q chains for qc 0 AND 1, plus v for tb 0..3,
            # interleaved by ci so each xT tile is consumed as soon as its
            # DMA lands. Accumulators live in the MAIN pools' tag rings (a
            # separate pool's release would barrier the first attention
            # writes on ALL ramp drain copies).
            rkq = [psS.tile([128, 1024], F32, tag="sps", name=f"rkq{i}")
                   for i in range(2)]
            rv = [psY.tile([128, 512], F32, tag="y0", name="rv0"),
                  psY.tile([128, 512], F32, tag="y1", name="rv1"),
                  psF.tile([128, 512], F32, tag="f", name="rv2"),
                  psF.tile([128, 512], F32, tag="f", name="rv3")]
            ramp_kq = [(kT[0][0], 0, 0), (qT[0][0], 1, 0),
                       (kT[0][1], 0, 1), (qT[0][1], 1, 1)]
            for ci in range(CI):
                for i, (dst, j, qc) in enumerate(ramp_kq):
                    nc.tensor.matmul(
                        rkq[i // 2][:, ts(i % 2, 512)],
                        w_sb[j][:, ts(ci, 128)],
                        xT_sb[ci][:, ts(qc, 512)],
                        start=(ci == 0), stop=(ci == CI - 1),
                    )
                for t in range(4):
                    nc.tensor.matmul(
                        rv[t][:, 0:256], xT_sb[ci][:, ts(t, 128)],
                        wv_sb[:, ts(ci, 256)],
                        start=(ci == 0), stop=(ci == CI - 1),
                    )
            # drain PSUM on ACT+DVE in parallel; unblock attn(0,0) S then AV
            nc.vector.tensor_copy(kT[0][0][:], rkq[0][:, 0:512])
            nc.scalar.copy(qT[0][0][:], rkq[0][:, 512:1024])
            v_copy(rv[0], 0, split=True)
            nc.vector.tensor_copy(kT[0][1][:], rkq[1][:, 0:512])
            nc.scalar.copy(qT[0][1][:], rkq[1][:, 512:1024])
            for t in range(1, 4):
                v_copy(rv[t], t, split=True)
            # ---- filler units ----
            def qk_chain(p, qc, which):
                dst = (kT if which == "k" else qT)[p][qc]
                j = 2 * p + (0 if which == "k" else 1)
                ps = psF.tile([128, 512], F32, tag="f", name="fqk")
                for ci in range(CI):
                    nc.tensor.matmul(
                        ps[:], w_sb[j][:, ts(ci, 128)],
                        xT_sb[ci][:, ts(qc, 512)],
                        start=(ci == 0), stop=(ci == CI - 1),
                    )
                nc.vector.tensor_copy(dst[:], ps[:])

            def v_chain(tb):
                psv = psF.tile([128, 256], F32, tag="f", name="fv")
                for ci in range(CI):
                    nc.tensor.matmul(
                        psv[:], xT_sb[ci][:, ts(tb, 128)],
                        wv_sb[:, ts(ci, 256)],
                        start=(ci == 0), stop=(ci == CI - 1),
                    )
                v_copy(psv, tb)

            ot_tiles = {}

            def proj_chunk(tb, cc):
                po = psF.tile([128, 512], F32, tag="f", name="fpo")
                for p in range(PAIRS):
                    nc.tensor.matmul(
                        po[:], yT[p][tb // 4][:, ts(tb % 4, 128)],
                        wp_sb[:, p * 1024 + cc * 512: p * 1024 + cc * 512 + 512],
                        start=(p == 0), stop=(p == PAIRS - 1),
                    )
                if cc == 0:
                    ot_tiles[tb] = pO.tile([128, 1024], F32, tag="ot", name="ot")
                ot = ot_tiles[tb]
                nc.vector.tensor_copy(ot[:, ts(cc, 512)], po[:])
                if cc == 1:
                    eng = (nc.scalar, nc.gpsimd, nc.sync)[tb % 3]
                    eng.dma_start(out_d[ts(tb, 128), :], ot[:])
                    del ot_tiles[tb]

            filler_queue = []

            def emit_filler(n=1):
                for _ in range(n):
                    if not filler_queue:
                        return
                    kind, args = filler_queue.pop(0)
                    if kind == "qk":
                        qk_chain(*args)
                    elif kind == "v":
                        v_chain(*args)
                    else:
                        proj_chunk(*args)

            # ---- attention for (pair, qc): AV lags S by one step ----
            def attn(p, qc, budget=0):
                nkb = 4 * qc + 4
                ypt = [psY.tile([128, 512], F32, tag=f"y{hh}", name=f"y{hh}")
                       for hh in (0, 1)]
                pts = {}

                def s_step(kb):
                    col = max(0, (kb - 4 * qc) * 128)
                    sps = psS.tile([128, 1024], F32, tag="sps", name="sps")
                    for hh in (0, 1):
                        off = hh * 64
                        nc.tensor.matmul(
                            sps[:, hh * 512 + col: hh * 512 + 512],
                            kT[p][kb // 4][off:off + 64, ts(kb % 4, 128)],
                            qT[p][qc][off:off + 64, col:512],
                            start=True, stop=True,
                        )
                    pt = pP.tile([128, 1024], P_DT, tag="pt", name="pt")
                    if col == 0:
                        nc.scalar.activation(pt[:], sps[:], AF.Exp)
                    else:
                        for hh in (0, 1):
                            nc.scalar.activation(
                                pt[:, hh * 512 + col: hh * 512 + 512],
                                sps[:, hh * 512 + col: hh * 512 + 512],
                                AF.Exp)
                    if kb >= 4 * qc:   # mask the diagonal 128-block
                        for hh in (0, 1):
                            nc.vector.tensor_mul(
                                pt[:, hh * 512 + col: hh * 512 + col + 128],
                                pt[:, hh * 512 + col: hh * 512 + col + 128],
                                mask_d[:],
                            )
                    pts[kb] = (pt, col)

                def av_step(kb):
                    pt, col = pts.pop(kb)
                    for hh in (0, 1):
                        h = 2 * p + hh
                        nc.tensor.matmul(
                            ypt[hh][:, col:512],
                            vt[kb][:, ts(h, 128)],
                            pt[:, hh * 512 + col: hh * 512 + 512],
                            start=(kb == 0), stop=(kb == nkb - 1),
                        )

                s_step(0)
                # spread `budget` fillers across steps, front-loaded, leaving
                # the last two steps clear so DVE/ACT drain by the boundary
                nfs = max(nkb - 2, 1)
                for kb in range(nkb):
                    if kb + 1 < nkb:
                        s_step(kb + 1)
                    if kb < nfs:
                        emit_filler(budget * (kb + 2) // (nfs + 1)
                                    - budget * (kb + 1) // (nfs + 1))
                    av_step(kb)
                # lazy normalize. Normal path: ONE [65,512] copy per head
                # (split DVE/ACT) frees the ypt bank ~0.7us after the last AV;
                # the l-extract / broadcast / reciprocal / multiply then run
                # off the critical path (the consuming proj trails by a qc).
                # Final qc: latency to yT gates the drain, so broadcast l via
                # a PE outer product (the PE is idle right then) and multiply
                # straight out of PSUM.
                last = (p == 1 and qc == NQC - 1)
                if not last:
                    stage = []
                    for hh in (0, 1):
                        st = pN.tile([65, 512], F32, tag=f"st{hh}", name="st")
                        (nc.vector.tensor_copy if hh == 0
                         else nc.scalar.copy)(st[:], ypt[hh][0:65, :])
                        stage.append(st)

                    def fin():
                        # deferred so DVE work emitted meanwhile (prologue
                        # chain copies) is not stuck behind the recip, which
                        # blocks in-order waiting for the gpsimd broadcast
                        for hh in (0, 1):
                            st = stage[hh]
                            l_sb = pN.tile([1, 512], F32, tag="l", name="l_sb")
                            nc.scalar.copy(l_sb[:], st[64:65, :])
                            lb = pN.tile([64, 512], F32, tag="lb", name="lb")
                            nc.gpsimd.partition_broadcast(lb[:], l_sb[:])
                            rl = pN.tile([64, 512], F32, tag="rl", name="rl")
                            nc.vector.reciprocal_approx_fast(rl[:], lb[:])
                            nc.vector.tensor_mul(
                                yT[p][qc][hh * 64: hh * 64 + 64, :],
                                st[0:64, :], rl[:],
                            )
                    return fin
                for hh in (0, 1):
                    l_sb = pN.tile([1, 512], F32, tag="l", name="l_sb")
                    nc.scalar.copy(l_sb[:], ypt[hh][64:65, :])
                    lb = pN.tile([64, 512], F32, tag="lb", name="lb")
                    nc.gpsimd.partition_broadcast(lb[:], l_sb[:])
                    rl = pN.tile([64, 512], F32, tag="rl", name="rl")
                    nc.vector.reciprocal_approx_fast(rl[:], lb[:])
                    nc.vector.tensor_mul(
                        yT[p][qc][hh * 64: hh * 64 + 64, :],
                        ypt[hh][0:64, :], rl[:],
                    )
                return None

            # ---- phase 1: pair 0 attention; fillers = v + pair-1 qk ----
            # queue order must respect deps: v(tb) before AV step kb=tb of
            # attn(0, tb//4); k1/q1(qc) anytime before attn(1, qc).
            filler_queue += [("qk", (1, 0, "k")), ("qk", (1, 0, "q")),
                             ("v", (4,))]                               # qc0: 3
            filler_queue += [("v", (5,)), ("v", (6,)), ("v", (7,)),
                             ("v", (8,)), ("v", (9,))]                  # qc1: 5
            filler_queue += [("v", (10,)), ("v", (11,)),
                             ("qk", (1, 1, "k")), ("qk", (1, 1, "q")),
                             ("v", (12,)), ("v", (13,))]                # qc2: 6
            filler_queue += [("v", (14,)), ("v", (15,)),
                             ("qk", (1, 2, "k")), ("qk", (1, 2, "q"))]  # qc3: 4
            p1_budget = [3, 5, 6, 4]

            fin = None
            for qc in range(NQC):
                if qc > 1:   # qc 0/1 chains were produced by the ramp
                    qk_chain(0, qc, "k")
                    qk_chain(0, qc, "q")
                if fin is not None:
                    fin()
                fin = attn(0, qc, p1_budget[qc])
            # phase-transition cover for attn(1,0)'s PSUM-bank reuse
            qk_chain(1, 3, "k")
            qk_chain(1, 3, "q")
            fin()

            # ---- phase 2: pair 1 attention; fillers = projection ----
            # budgets keep 2 chunks of each qc for the NEXT attention so the
            # first pops at a boundary never wait on the just-written yT
            p2_budget = [0, 6, 8, 8]
            for qc in range(NQC):
                fin = attn(1, qc, p2_budget[qc])
                if fin is not None:
                    fin()
                filler_queue += [("proj", (tb, cc))
                                 for tb in range(4 * qc, 4 * qc + 4)
                                 for cc in range(2)]
            emit_filler(len(filler_queue))



_NC_CACHE = None


def _build():
    global _NC_CACHE
    if _NC_CACHE is not None:
        return _NC_CACHE
    nc = bacc.Bacc("TRN2", target_bir_lowering=False, debug=False,
                   num_devices=N_CORES)
    xT_d = nc.dram_tensor("xT", [C, T], IO_DT, kind="ExternalInput")
    w1_d = nc.dram_tensor("w1", [128, 4096], IO_DT, kind="ExternalInput")
    wv_d = nc.dram_tensor("wv", [128, 2048], IO_DT, kind="ExternalInput")
    wp_d = nc.dram_tensor("wp", [128, 2048], IO_DT, kind="ExternalInput")
    out_d = nc.dram_tensor("out", [T, C], F32, kind="ExternalOutput")

    with tile.TileContext(nc) as tc:
        _emit(tc, nc, xT_d, w1_d, wv_d, wp_d, out_d)
    nc.compile()
    _NC_CACHE = nc
    return nc


def _pack_pair(m):
    # [1024, 128] -> lhsT chunks layout [128, 8*128]
    return np.ascontiguousarray(
        m.reshape(CI, 128, 128).transpose(1, 0, 2).reshape(128, 1024))


def _io_np(a):
    import ml_dtypes
    return np.ascontiguousarray(a.astype(ml_dtypes.bfloat16))


def _in_maps(x, w_attn, w_proj):
    x = np.asarray(x, dtype=np.float32)
    w_attn = np.asarray(w_attn, dtype=np.float32)
    w_proj = np.asarray(w_proj, dtype=np.float32)
    xT = [_io_np(x[b].T) for b in range(B)]
    maps = []
    for core in range(N_CORES):
        b, g = core // HPC, core % HPC
        cols = slice(g * 256, (g + 1) * 256)
        wk_full = w_attn[:, 0 * C:1 * C][:, cols]
        wq_full = w_attn[:, 1 * C:2 * C][:, cols] * np.float32(1.0 / np.sqrt(HD))
        wv_full = w_attn[:, 2 * C:3 * C][:, cols]
        w1 = np.concatenate(
            [_pack_pair(m[:, p * 128:(p + 1) * 128])
             for p in range(PAIRS) for m in (wk_full, wq_full)], axis=1)
        wv_in = wv_full.reshape(CI, 128, 256).transpose(1, 0, 2).reshape(128, 2048)
        wp_in = (w_proj[g * 256:(g + 1) * 256, :]
                 .reshape(PAIRS, 128, 1024).transpose(1, 0, 2).reshape(128, 2048))
        maps.append({"xT": xT[b], "w1": _io_np(w1),
                     "wv": _io_np(wv_in), "wp": _io_np(wp_in)})
    return maps


def _assemble(results, b_proj):
    b_proj = np.asarray(b_proj, dtype=np.float32)
    out = np.zeros((B, T, C), dtype=np.float32)
    for core in range(N_CORES):
        out[core // HPC] += results[core]["out"]
    out += b_proj[None, None, :]
    return out


def kernel(x, w_attn, w_proj, b_proj):
    nc = _build()
    maps = _in_maps(x, w_attn, w_proj)
    res = run_bass_kernel_spmd(nc, maps, list(range(N_CORES)))
    return _assemble(res.results, b_proj)


def kernel_traced(x, w_attn, w_proj, b_proj):
    """Like kernel() but with NTFF tracing; returns (out, BassKernelResults)."""
    nc = _build()
    maps = _in_maps(x, w_attn, w_proj)
    res = run_bass_kernel_spmd(nc, maps, list(range(N_CORES)), trace=True)
    return _assemble(res.results, b_proj), res
